# revision 33
# baseline (speedup 1.0000x reference)
"""Distributed LlamaAttention (B=2, S=2048, H=2048, 16 heads) on one TRN2 chip.

Sharding: tensor-parallel over heads — core c owns heads (2c, 2c+1).
  * q/k projections: out-feature (head) slices, produced transposed [d, tok]
  * v projection: operand-swapped (hs stationary) to produce natural [tok, d]
  * attention computed with TRANSPOSED scores sT[k, q] (k on partitions), so
    softmax weights come out already in the layout the AV matmul needs —
    no PE transposes at all. Rowsums: DVE accumulates a per-partition
    colsum of the exp tiles (bf16, 2x rate), one [128,1]-ones matmul
    reduces it across partitions, a K=1 matmul broadcasts it back, and a
    128-lane fast reciprocal + multiply normalizes. The whole chain is
    software-pipelined one group behind the PE.
  * o-projection: row-parallel (in-feature slices of wo) -> per-core partials
  * unshard: host sums the 8 partial outputs

All matmuls run in bf16 (TensorE 1 cycle/row) with f32 PSUM accumulation.
Emission order interleaves phases (batch-1 projections and o-projection
tiles are woven between and *inside* attention groups) so the PE always
has independent work while ScalarE runs the exps.

Self-contained: hardcodes all shapes; no sibling imports.
"""

import math
from collections import deque

import numpy as np
import ml_dtypes

B, S, HIDDEN, NH, HD = 2, 2048, 2048, 16, 128
N_CORES = 8
HPC = NH // N_CORES          # heads per core = 2
M = HPC * HD                 # per-core projection width = 256
T = B * S                    # 4096 tokens
P = 128                      # partitions
TCH = 512                    # free-dim chunk
KI = HIDDEN // P             # 16 contraction tiles for projections
QT = S // P                  # 16 token tiles per batch elem
JQ = S // TCH                # 4 query chunks per batch elem
BF16 = ml_dtypes.bfloat16

_nc_cache = {}


def _build_nc():
    import concourse.bacc as bacc
    import concourse.mybir as mybir
    from concourse import tile
    from contextlib import ExitStack

    bf = mybir.dt.bfloat16
    f32 = mybir.dt.float32
    AF = mybir.ActivationFunctionType

    nc = bacc.Bacc("TRN2", target_bir_lowering=False, debug=False)

    hsT = nc.dram_tensor("hsT", [HIDDEN, T], bf, kind="ExternalInput").ap()
    wqT = nc.dram_tensor("wqT", [HIDDEN, M], bf, kind="ExternalInput").ap()
    wkT = nc.dram_tensor("wkT", [HIDDEN, M], bf, kind="ExternalInput").ap()
    wvT = nc.dram_tensor("wvT", [HIDDEN, M], bf, kind="ExternalInput").ap()
    woT = nc.dram_tensor("woT", [M, HIDDEN], bf, kind="ExternalInput").ap()
    msk = nc.dram_tensor("mask", [P, P], f32, kind="ExternalInput").ap()
    onc = nc.dram_tensor("onesc", [P, 1], bf, kind="ExternalInput").ap()
    onr = nc.dram_tensor("onesr", [1, P], bf, kind="ExternalInput").ap()
    out = nc.dram_tensor("out", [T, HIDDEN], bf, kind="ExternalOutput").ap()

    hsT_r = hsT.rearrange("(i p) t -> p i t", p=P)      # [128, 16, 4096]
    out_r = out.rearrange("(n p) o -> p n o", p=P)      # [128, 32, 2048]

    inv_sqrt_d = 1.0 / math.sqrt(HD)

    with tile.TileContext(nc) as tc, ExitStack() as ctx:
        const = ctx.enter_context(tc.tile_pool(name="const", bufs=1))
        qkv = ctx.enter_context(tc.tile_pool(name="qkv", bufs=1))
        hsp = ctx.enter_context(tc.tile_pool(name="hsp", bufs=2))
        expp = ctx.enter_context(tc.tile_pool(name="expp", bufs=2))
        csp = ctx.enter_context(tc.tile_pool(name="csp", bufs=2))
        rcp = ctx.enter_context(tc.tile_pool(name="rcp", bufs=2))
        rbp = ctx.enter_context(tc.tile_pool(name="rbp", bufs=2))
        opl = ctx.enter_context(tc.tile_pool(name="opl", bufs=4))
        mm = ctx.enter_context(tc.tile_pool(name="mm", bufs=3, space="PSUM"))
        sps = ctx.enter_context(tc.tile_pool(name="sps", bufs=2, space="PSUM"))
        avp = ctx.enter_context(tc.tile_pool(name="avp", bufs=2, space="PSUM"))
        rps = ctx.enter_context(tc.tile_pool(name="rps", bufs=1, space="PSUM"))

        # --- weights / constants; wq + first hs chunk interleaved first ---
        wq_sb = const.tile([P, KI, M], bf, name="wq_sb")
        wqT_r = wqT.rearrange("(i p) m -> p i m", p=P)
        hs_tiles = {}
        hs0 = hsp.tile([P, KI, TCH], bf, tag="hs", name="hs0")
        hs_tiles[0] = hs0
        for q in range(4):
            nc.sync.dma_start(wq_sb[:, 4 * q:4 * q + 4, :], wqT_r[:, 4 * q:4 * q + 4, :])
            nc.sync.dma_start(
                hs0[:, 4 * q:4 * q + 4, :], hsT_r[:, 4 * q:4 * q + 4, 0:TCH]
            )

        def emit_hs_dma(j):
            hs_t = hsp.tile([P, KI, TCH], bf, tag="hs", name=f"hs{j}")
            for q in range(4):
                nc.sync.dma_start(
                    hs_t[:, 4 * q:4 * q + 4, :],
                    hsT_r[:, 4 * q:4 * q + 4, j * TCH:(j + 1) * TCH],
                )
            hs_tiles[j] = hs_t

        wk_sb = const.tile([P, KI, M], bf, name="wk_sb")
        wv_sb = const.tile([P, KI, M], bf, name="wv_sb")
        wkT_r = wkT.rearrange("(i p) m -> p i m", p=P)
        wvT_r = wvT.rearrange("(i p) m -> p i m", p=P)
        nc.sync.dma_start(wk_sb[:, 0:8, :], wkT_r[:, 0:8, :])
        nc.sync.dma_start(wk_sb[:, 8:16, :], wkT_r[:, 8:16, :])
        nc.sync.dma_start(wv_sb[:, 0:8, :], wvT_r[:, 0:8, :])
        nc.sync.dma_start(wv_sb[:, 8:16, :], wvT_r[:, 8:16, :])
        wo_sb = const.tile([P, HPC, HIDDEN], bf, name="wo_sb")
        nc.sync.dma_start(wo_sb[:], woT.rearrange("(mt p) o -> p mt o", p=P))
        msk_sb = const.tile([P, P], f32, name="msk_sb")
        nc.sync.dma_start(msk_sb[:], msk)
        onc_sb = const.tile([P, 1], bf, name="onc_sb")
        nc.sync.dma_start(onc_sb[:], onc)
        onr_sb = const.tile([1, P], bf, name="onr_sb")
        nc.sync.dma_start(onr_sb[:], onr)

        # --- persistent activations ---
        qT_b = [qkv.tile([P, HPC, S], bf, tag=f"qT{b}", name=f"qT{b}") for b in range(B)]
        kT_b = [qkv.tile([P, HPC, S], bf, tag=f"kT{b}", name=f"kT{b}") for b in range(B)]
        vn_b = [qkv.tile([P, QT, M], bf, tag=f"vn{b}", name=f"vn{b}") for b in range(B)]
        cxT_b = [qkv.tile([P, HPC, S], bf, tag=f"cxT{b}", name=f"cxT{b}") for b in range(B)]

        # --- filler queue: (credits, fn) units the PE can chew on any time ---
        filler_q = deque()
        credit = {"c": 0}

        def pop_filler(n=1):
            credit["c"] += n
            while filler_q and credit["c"] >= filler_q[0][0]:
                c, fn = filler_q.popleft()
                credit["c"] -= c
                fn()

        # ---- QKV projection emission units (6 per 512-token chunk) ----
        def qk_block(b, j4, w_sb, dst, mt):
            hs_t = hs_tiles[b * JQ + j4]
            ps = mm.tile([P, TCH], f32, tag="mm", name=f"mmqk{b}{j4}{mt}")
            for i in range(KI):
                nc.tensor.matmul(
                    ps[:],
                    w_sb[:, i, mt * P:(mt + 1) * P],
                    hs_t[:, i, :],
                    start=(i == 0),
                    stop=(i == KI - 1),
                )
            nc.vector.tensor_copy(dst[:, mt, j4 * TCH:(j4 + 1) * TCH], ps[:])

        def v_block(b, j4, tsub):
            hs_t = hs_tiles[b * JQ + j4]
            ps = mm.tile([P, M], f32, tag="mm", name=f"mmv{b}{j4}{tsub}")
            for i in range(KI):
                nc.tensor.matmul(
                    ps[:],
                    hs_t[:, i, tsub * P:(tsub + 1) * P],
                    wv_sb[:, i, :],
                    start=(i == 0),
                    stop=(i == KI - 1),
                )
            nc.vector.tensor_copy(vn_b[b][:, j4 * 4 + tsub, :], ps[:])

        def qkv_units(b):
            units = []
            for j4 in range(JQ):
                j = b * JQ + j4
                def u0(b=b, j4=j4):
                    qk_block(b, j4, wq_sb, qT_b[b], 0)
                def u1(b=b, j4=j4):
                    qk_block(b, j4, wq_sb, qT_b[b], 1)
                def u2(b=b, j4=j4):
                    qk_block(b, j4, wk_sb, kT_b[b], 0)
                def u3(b=b, j4=j4):
                    qk_block(b, j4, wk_sb, kT_b[b], 1)
                def u4(b=b, j4=j4):
                    v_block(b, j4, 0)
                    v_block(b, j4, 1)
                def u5(b=b, j4=j4, j=j):
                    v_block(b, j4, 2)
                    v_block(b, j4, 3)
                    if j + 1 < 2 * JQ:
                        emit_hs_dma(j + 1)
                units += [u0, u1, u2, u3, u4, u5]
            return units

        # ---- o-projection micro-units (per 512-wide output chunk) ----
        orow_state = {}

        def oproj_oc(b, tt, oc, vec=False):
            if oc == 0:
                orow_state[(b, tt)] = opl.tile(
                    [P, HIDDEN], bf, tag="orow", name=f"orow{b}{tt}"
                )
            orow = orow_state[(b, tt)]
            ps = mm.tile([P, TCH], f32, tag="mm", name=f"mmo{b}{tt}{oc}")
            for mt in range(HPC):
                nc.tensor.matmul(
                    ps[:],
                    cxT_b[b][:, mt, tt * P:(tt + 1) * P],
                    wo_sb[:, mt, oc * TCH:(oc + 1) * TCH],
                    start=(mt == 0),
                    stop=(mt == HPC - 1),
                )
            if oc == 0:
                nc.scalar.copy(orow[:, oc * TCH:(oc + 1) * TCH], ps[:])
            else:
                nc.vector.tensor_copy(orow[:, oc * TCH:(oc + 1) * TCH], ps[:])
            nc.sync.dma_start(
                out_r[:, b * QT + tt, oc * TCH:(oc + 1) * TCH],
                orow[:, oc * TCH:(oc + 1) * TCH],
            )

        def queue_oproj(b, tts, vec=False):
            for tt in tts:
                for oc in range(HIDDEN // TCH):
                    filler_q.append(
                        (1, lambda b=b, tt=tt, oc=oc: oproj_oc(b, tt, oc, vec=vec))
                    )

        # ---- attention group: transposed-scores flash block ----
        def attn_group_gen(b, h, jq):
            ktmax = 4 * jq + 4
            kts = list(range(4 * jq, 4 * jq + 4)) + list(range(0, 4 * jq))
            expT = expp.tile([P, QT, TCH], bf, tag="exp", name=f"expT{b}{h}{jq}")
            av_ps = avp.tile([P, TCH], f32, tag="av", name=f"av{b}{h}{jq}")
            colsum = csp.tile([P, TCH], bf, tag="cs", name=f"cs{b}{h}{jq}")
            colsum2 = csp.tile([P, TCH], bf, tag="cs2", name=f"cs2{b}{h}{jq}")

            def flush(idx, kt, off):
                nc.tensor.matmul(
                    av_ps[:, off:TCH],
                    vn_b[b][:, kt, h * P:(h + 1) * P],
                    expT[:, kt, off:TCH],
                    start=(idx == 0),
                    stop=(idx == ktmax - 1),
                )

            adds = deque()

            def emit_add():
                # two independent accumulator chains (DVE + GpSimd), merged at end
                idx, kt, off = adds.popleft()
                if idx == 0:
                    nc.vector.tensor_copy(colsum[:], expT[:, kt, :])
                elif idx == 1:
                    nc.gpsimd.memset(colsum2[:, 0:off], 0.0)
                    nc.gpsimd.tensor_copy(colsum2[:, off:TCH], expT[:, kt, off:TCH])
                elif idx % 2 == 0:
                    nc.vector.tensor_add(
                        colsum[:, off:TCH], colsum[:, off:TCH], expT[:, kt, off:TCH]
                    )
                else:
                    nc.gpsimd.tensor_add(
                        colsum2[:, off:TCH], colsum2[:, off:TCH], expT[:, kt, off:TCH]
                    )

            pend = []
            for idx, kt in enumerate(kts):
                off = (kt - 4 * jq) * P if kt >= 4 * jq else 0
                s_ps = sps.tile([P, TCH], f32, tag="s", name=f"s{b}{h}{jq}{kt}")
                nc.tensor.matmul(
                    s_ps[:, off:TCH],
                    kT_b[b][:, h, kt * P:(kt + 1) * P],
                    qT_b[b][:, h, jq * TCH + off:(jq + 1) * TCH],
                    start=True,
                    stop=True,
                )
                if kt >= 4 * jq:
                    nc.vector.tensor_add(
                        s_ps[:, off:off + P], s_ps[:, off:off + P], msk_sb[:]
                    )
                nc.scalar.activation(
                    expT[:, kt, off:TCH], s_ps[:, off:TCH], AF.Exp, scale=inv_sqrt_d
                )
                pend.append((idx, kt, off))
                adds.append((idx, kt, off))
                if len(pend) > 2:
                    flush(*pend.pop(0))
                if idx >= 2:
                    pop_filler(1)
                if idx >= 4:
                    emit_add()
                if idx == 3:
                    yield None
                if idx == 5:
                    yield None
            for args in pend:
                flush(*args)
            while adds:
                emit_add()
            nc.vector.tensor_add(colsum[:], colsum[:], colsum2[:])
            if ktmax <= 5:
                yield None
            yield (colsum, av_ps)

        def emit_fin2a(p):
            # reduce the colsum across partitions: one ones-matmul per group
            b, h, jq = p["b"], p["h"], p["jq"]
            r_ps = rps.tile([1, TCH], f32, tag="r", name=f"r{b}{h}{jq}")
            nc.tensor.matmul(
                r_ps[0:1, :], onc_sb[:, 0:1], p["colsum"][:], start=True, stop=True
            )
            r_bf = rcp.tile([1, TCH], bf, tag="rcb", name=f"rcb{b}{h}{jq}")
            nc.vector.tensor_copy(r_bf[0:1, :], r_ps[0:1, :])
            p["r_bf"] = r_bf

        def emit_fin2b(p):
            # broadcast rowsums across partitions (K=1 matmul), then 128-lane
            # fast reciprocal and the normalization multiply
            b, h, jq = p["b"], p["h"], p["jq"]
            rb_ps = rps.tile([P, TCH], f32, tag="r", name=f"rb{b}{h}{jq}")
            nc.tensor.matmul(
                rb_ps[:], onr_sb[0:1, :], p["r_bf"][0:1, :], start=True, stop=True
            )
            rb_sb = rbp.tile([P, TCH], f32, tag="rbs", name=f"rbs{b}{h}{jq}")
            nc.vector.reciprocal_approx_fast(rb_sb[:], rb_ps[:])
            nc.vector.tensor_mul(
                cxT_b[b][:, h, jq * TCH:(jq + 1) * TCH], p["av_ps"][:], rb_sb[:]
            )

        prev = {"p": None}

        def run_group(b, h, jq):
            gen = attn_group_gen(b, h, jq)
            next(gen)                      # diagonal tiles emitted
            if prev["p"] is not None:
                emit_fin2a(prev["p"])
            next(gen)
            if prev["p"] is not None:
                emit_fin2b(prev["p"])
            colsum, av_ps = next(gen)
            prev["p"] = {"b": b, "h": h, "jq": jq, "colsum": colsum, "av_ps": av_ps}

        # ---- drive ----
        for u in qkv_units(0):
            u()
        u1s = qkv_units(1)
        for u in u1s[:18]:                 # chunks 0-2 fill batch-0 attention
            filler_q.append((4, u))
        for h in range(HPC):
            for jq in range(JQ):
                run_group(0, h, jq)
                pop_filler(3)
        while filler_q:
            pop_filler(4)
        for u in u1s[18:]:                 # chunk 3 held back for b1 region
            filler_q.append((4, u))
        queue_oproj(0, range(QT))
        for jq in range(JQ):
            if jq >= 2:
                queue_oproj(1, range(4 * (jq - 2), 4 * (jq - 1)))
            run_group(1, 0, jq)
            pop_filler(2)
            if jq == 3:
                queue_oproj(1, range(8, 12), vec=True)
            run_group(1, 1, jq)
            pop_filler(2)
        pop_filler(2)
        emit_fin2a(prev["p"])
        pop_filler(2)
        emit_fin2b(prev["p"])              # final group's normalization
        queue_oproj(1, range(12, QT), vec=True)
        while filler_q:
            pop_filler(4)

    nc.compile()
    return nc


def get_nc():
    if "nc" not in _nc_cache:
        _nc_cache["nc"] = _build_nc()
    return _nc_cache["nc"]


def make_in_maps(hidden_states, wq, wk, wv, wo):
    hs = np.asarray(hidden_states, dtype=np.float32).reshape(T, HIDDEN)
    hsT = np.ascontiguousarray(hs.T).astype(BF16)
    # sT orientation: element (k, q) invalid (masked) when q < k
    mask = np.tril(np.full((P, P), -1e9, dtype=np.float32), -1)
    onesc = np.ones((P, 1), dtype=np.float32).astype(BF16)
    onesr = np.ones((1, P), dtype=np.float32).astype(BF16)
    wq = np.asarray(wq, dtype=np.float32)
    wk = np.asarray(wk, dtype=np.float32)
    wv = np.asarray(wv, dtype=np.float32)
    wo = np.asarray(wo, dtype=np.float32)
    in_maps = []
    for c in range(N_CORES):
        sl = slice(c * M, (c + 1) * M)
        in_maps.append({
            "hsT": hsT,
            "wqT": np.ascontiguousarray(wq[sl, :].T).astype(BF16),
            "wkT": np.ascontiguousarray(wk[sl, :].T).astype(BF16),
            "wvT": np.ascontiguousarray(wv[sl, :].T).astype(BF16),
            "woT": np.ascontiguousarray(wo[:, sl].T).astype(BF16),
            "mask": mask,
            "onesc": onesc,
            "onesr": onesr,
        })
    return in_maps


def kernel(hidden_states, wq, wk, wv, wo):
    from concourse.bass_utils import run_bass_kernel_spmd

    nc = get_nc()
    in_maps = make_in_maps(hidden_states, wq, wk, wv, wo)
    res = run_bass_kernel_spmd(nc, in_maps, core_ids=list(range(N_CORES)))
    acc = np.zeros((T, HIDDEN), dtype=np.float32)
    for r in res.results:
        acc += np.asarray(r["out"]).astype(np.float32)
    return acc.reshape(B, S, HIDDEN)
